# revision 8
# baseline (speedup 1.0000x reference)
"""CRF loss (BertCrf) kernel for 8 Trainium2 NeuronCores (Bass/Tile).

Strategy: the partition-function scan is the only heavy part. The batch
(B=256) is split into 4 groups of 64 sequences; for each group one core runs
the forward recursion over t in [0,256) and a second core runs the backward
recursion over t in [256,512) (expressed as a forward scan over time-reversed
features with the transposed transition matrix), halving the sequential chain.
All 8 cores execute the same program; only their input data differs.

Per-core math (linear space, state [L=128 part, 64 batch], bf16):
    E_t = exp(feat_t - C);  Y_0 = expSE * E_0;  Y_k = (M^T @ Y_{k-1}) * E_k
with a per-column power-of-2 rescale every 16 steps (column-sum matmul ->
fp32 exponent bit trick -> K=1 broadcast matmul -> multiply).  The host glues
the two halves:  log_den = log(sum_j Yf_j * (M^T Yb)_j) + 512*C - sum(log s).
The gold-path score (pure gathers) and final mean run on host in numpy.
"""

import numpy as np
import ml_dtypes

B, S, L = 256, 512, 128
NC = 8
TH = 256          # time steps per core
C_SHIFT = 4.5
RESC = 16
NRESC = (TH - 1) // RESC  # 15 rescale rounds (k = 16, 32, ..., 240)
bf16 = ml_dtypes.bfloat16

_cache = {}


# --------------------------------------------------------------------------
# device program
# --------------------------------------------------------------------------

def _build_bass():
    import concourse.bass as bass
    import concourse.mybir as mybir
    from concourse import tile

    f32, i32, bf = mybir.dt.float32, mybir.dt.int32, mybir.dt.bfloat16
    Exp = mybir.ActivationFunctionType.Exp
    Alu = mybir.AluOpType

    nc = bass.Bass()
    feats = nc.declare_dram_parameter("feats", [64, TH, L], f32, isOutput=False)
    m_in = nc.declare_dram_parameter("m", [L, L], bf, isOutput=False)
    se_in = nc.declare_dram_parameter("expse", [L, 1], f32, isOutput=False)
    id_in = nc.declare_dram_parameter("ident", [L, L], f32, isOutput=False)
    y_out = nc.declare_dram_parameter("y", [L, 64], bf, isOutput=True)
    py_out = nc.declare_dram_parameter("py", [L, 64], f32, isOutput=True)
    mx_out = nc.declare_dram_parameter("mx", [1, 1024], f32, isOutput=True)

    with tile.TileContext(nc) as tc:
        with tc.tile_pool(name="const", bufs=1) as cpool, \
             tc.tile_pool(name="ef", bufs=1) as efpool, \
             tc.tile_pool(name="stage", bufs=8) as stpool, \
             tc.tile_pool(name="yp", bufs=3) as ypool, \
             tc.tile_pool(name="small", bufs=2) as smpool, \
             tc.tile_pool(name="psT", bufs=2, space="PSUM") as psT, \
             tc.tile_pool(name="psP", bufs=2, space="PSUM") as psP, \
             tc.tile_pool(name="psC", bufs=1, space="PSUM") as psC, \
             tc.tile_pool(name="psR", bufs=1, space="PSUM") as psR:

            m_sb = cpool.tile([L, L], bf)
            nc.sync.dma_start(out=m_sb[:], in_=m_in[:])
            id_sb = cpool.tile([L, L], f32)
            nc.sync.dma_start(out=id_sb[:], in_=id_in[:])
            se_sb = cpool.tile([L, 1], f32)
            nc.sync.dma_start(out=se_sb[:], in_=se_in[:])
            ones_col = cpool.tile([L, 1], bf)
            nc.vector.memset(ones_col[:], 1.0)
            ones_row = cpool.tile([1, L], f32)
            nc.vector.memset(ones_row[:], 1.0)
            nbias = cpool.tile([L, 1], f32)
            nc.vector.memset(nbias[:], -C_SHIFT)
            c_and = cpool.tile([1, 64], i32)
            nc.vector.memset(c_and[:], 0x7F800000)
            c_base = cpool.tile([1, 1], f32)
            nc.vector.memset(c_base[:], float(0x7F000000))
            c_neg1 = cpool.tile([1, 1], f32)
            nc.vector.memset(c_neg1[:], -1.0)
            mx_sb = cpool.tile([1, 1024], f32)
            nc.vector.memset(mx_sb[:], 1.0)

            # E tensors, one per 128-step block: [j, t, b] so the scan reads
            # EF[:, k, :] contiguously.
            efs = [efpool.tile([L, 128, 64], f32, tag=f"ef{g}", name=f"ef{g}")
                   for g in range(2)]

            def build_block(g):
                for b0 in range(0, 64, 4):
                    pt = psT.tile([L, 4, 128], f32, tag="pt")
                    for i in range(4):
                        b = b0 + i
                        ft = stpool.tile([L, 128], f32, tag="ft")
                        nc.sync.dma_start(
                            out=ft[:], in_=feats[b, 128 * g:128 * (g + 1), :])
                        nc.tensor.transpose(
                            out=pt[:, i, :], in_=ft[:], identity=id_sb[:])
                    dst = efs[g][:, :, b0:b0 + 4].rearrange("j t b -> j b t")
                    nc.scalar.activation(
                        out=dst, in_=pt[:], func=Exp,
                        bias=nbias[:, 0:1], scale=1.0)

            def scan_range(k0, k1, y):
                for k in range(k0, k1):
                    g, kk = divmod(k, 128)
                    p = psP.tile([L, 64], f32, tag="p")
                    nc.tensor.matmul(out=p[:], lhsT=m_sb[:], rhs=y[:],
                                     start=True, stop=True)
                    y = ypool.tile([L, 64], bf, tag="y")
                    nc.vector.tensor_tensor(
                        out=y[:], in0=p[:], in1=efs[g][:, kk, :], op=Alu.mult)
                    if k % RESC == 0:
                        r = k // RESC - 1
                        csum = psC.tile([1, 64], f32, tag="cs")
                        nc.tensor.matmul(out=csum[:], lhsT=ones_col[:],
                                         rhs=y[:], start=True, stop=True)
                        eb = smpool.tile([1, 64], i32, tag="eb")
                        nc.vector.tensor_tensor(
                            out=eb[:], in0=csum[:].bitcast(i32),
                            in1=c_and[:], op=Alu.bitwise_and)
                        sr = smpool.tile([1, 64], i32, tag="sr")
                        nc.vector.tensor_scalar(
                            out=sr[:], in0=eb[:], scalar1=c_neg1[:],
                            scalar2=c_base[:], op0=Alu.mult, op1=Alu.add)
                        rbp = psR.tile([L, 64], f32, tag="rb")
                        nc.tensor.matmul(out=rbp[:], lhsT=ones_row[:],
                                         rhs=sr[:].bitcast(f32),
                                         start=True, stop=True)
                        y2 = ypool.tile([L, 64], bf, tag="y")
                        nc.vector.tensor_tensor(
                            out=y2[:], in0=y[:], in1=rbp[:], op=Alu.mult)
                        nc.scalar.copy(
                            out=mx_sb[0:1, 64 * r:64 * (r + 1)], in_=csum[:])
                        y = y2
                return y

            build_block(0)
            y0 = ypool.tile([L, 64], bf, tag="y")
            nc.vector.tensor_scalar(
                out=y0[:], in0=efs[0][:, 0, :], scalar1=se_sb[:, 0:1],
                scalar2=None, op0=Alu.mult)
            y = scan_range(1, 128, y0)
            build_block(1)
            y = scan_range(128, TH, y)

            pfin = psP.tile([L, 64], f32, tag="p")
            nc.tensor.matmul(out=pfin[:], lhsT=m_sb[:], rhs=y[:],
                             start=True, stop=True)
            py_sb = stpool.tile([L, 64], f32, tag="pyo")
            nc.scalar.copy(out=py_sb[:], in_=pfin[:])
            nc.sync.dma_start(out=y_out[:], in_=y[:])
            nc.sync.dma_start(out=py_out[:], in_=py_sb[:])
            nc.sync.dma_start(out=mx_out[:], in_=mx_sb[:])
    return nc


# --------------------------------------------------------------------------
# cached PJRT runner (one jit, reused across calls)
# --------------------------------------------------------------------------

def _get_exec():
    if "exec" in _cache:
        return _cache["exec"]
    import jax
    from jax.sharding import Mesh, PartitionSpec
    try:
        from jax.experimental.shard_map import shard_map
    except ImportError:  # newer jax
        from jax.shard_map import shard_map
    from concourse import bass2jax
    import concourse.mybir as mybir

    nc = _build_bass()
    bass2jax.install_neuronx_cc_hook()

    partition_name = (nc.partition_id_tensor.name
                      if nc.partition_id_tensor else None)
    in_names, out_names, out_avals, out_shapes = [], [], [], []
    for alloc in nc.m.functions[0].allocations:
        if not isinstance(alloc, mybir.MemoryLocationSet):
            continue
        name = alloc.memorylocations[0].name
        if alloc.kind == "ExternalInput":
            if name != partition_name:
                in_names.append(name)
        elif alloc.kind == "ExternalOutput":
            out_names.append(name)
            shape = tuple(alloc.tensor_shape)
            dtype = mybir.dt.np(alloc.dtype)
            out_avals.append(jax.core.ShapedArray(shape, dtype))
            out_shapes.append((shape, dtype))
    n_params = len(in_names)
    all_in = list(in_names) + list(out_names)
    if partition_name is not None:
        all_in.append(partition_name)
    donate = tuple(range(n_params, n_params + len(out_names)))

    def _body(*args):
        operands = list(args)
        if partition_name is not None:
            operands.append(bass2jax.partition_id_tensor())
        outs = bass2jax._bass_exec_p.bind(
            *operands,
            out_avals=tuple(out_avals),
            in_names=tuple(all_in),
            out_names=tuple(out_names),
            lowering_input_output_aliases=(),
            sim_require_finite=True,
            sim_require_nnan=True,
            nc=nc,
        )
        return tuple(outs)

    devices = jax.devices()[:NC]
    assert len(devices) == NC, f"need {NC} devices, have {len(jax.devices())}"
    mesh = Mesh(np.asarray(devices), ("core",))
    n_io = n_params + len(out_names)
    sharded = jax.jit(
        shard_map(_body, mesh=mesh,
                  in_specs=(PartitionSpec("core"),) * n_io,
                  out_specs=(PartitionSpec("core"),) * len(out_names),
                  check_rep=False),
        donate_argnums=donate, keep_unused=True)
    _cache["exec"] = (sharded, in_names, out_names, out_shapes)
    return _cache["exec"]


# --------------------------------------------------------------------------
# host side
# --------------------------------------------------------------------------

def _pow2_recip(x):
    """2^(127-E) for fp32 x>0 — must match the device bit trick exactly."""
    bits = np.ascontiguousarray(x, dtype=np.float32).view(np.uint32)
    ebits = bits & np.uint32(0x7F800000)
    return (np.uint32(0x7F000000) - ebits).view(np.float32)


def _log_num_host(features, start, end, transitions, labels):
    labs = labels.astype(np.int64)
    labs = np.where(labs == -100, 0, labs)
    emit = np.take_along_axis(features, labs[:, :, None], axis=2)[..., 0]
    trs = transitions[labs[:, :-1], labs[:, 1:]]
    return (start[labs[:, 0]].astype(np.float64) + emit[:, 0]
            + (trs.astype(np.float64) + emit[:, 1:]).sum(axis=1)
            + end[labs[:, -1]])


def _prep_concat(features, start, end, transitions):
    """Concatenated (8*rows, ...) input arrays, core-major along axis 0."""
    expT = np.exp(transitions.astype(np.float32))
    m_f = expT.astype(bf16)
    m_b = np.ascontiguousarray(expT.T).astype(bf16)

    feats_all = np.empty((NC * 64, TH, L), np.float32)
    for c in range(4):
        sl = slice(64 * c, 64 * c + 64)
        feats_all[64 * c:64 * c + 64] = features[sl, :TH]
        feats_all[64 * (c + 4):64 * (c + 5)] = features[sl, S - 1:TH - 1:-1]
    m_all = np.empty((NC * L, L), bf16)
    se_all = np.empty((NC * L, 1), np.float32)
    es = np.exp(start.astype(np.float32))[:, None]
    ee = np.exp(end.astype(np.float32))[:, None]
    for c in range(NC):
        m_all[L * c:L * (c + 1)] = m_f if c < 4 else m_b
        se_all[L * c:L * (c + 1)] = es if c < 4 else ee
    ident_all = np.tile(np.eye(L, dtype=np.float32), (NC, 1))
    return {"feats": feats_all, "m": m_all, "expse": se_all,
            "ident": ident_all}


def _run_device(features, start, end, transitions):
    sharded, in_names, out_names, out_shapes = _get_exec()
    in_map = _prep_concat(features, start, end, transitions)
    zeros = [np.zeros((NC * sh[0], *sh[1:]), dt) for sh, dt in out_shapes]
    outs = sharded(*[in_map[n] for n in in_names], *zeros)
    res = {}
    for i, name in enumerate(out_names):
        sh, dt = out_shapes[i]
        res[name] = np.asarray(outs[i]).reshape(NC, *sh)
    return res


def _combine(res):
    y_all = res["y"].astype(np.float64)        # [NC, L, 64]
    py_all = res["py"].astype(np.float64)
    mx_all = res["mx"]                          # [NC, 1, 1024] fp32
    den = np.empty(B)
    for c in range(4):
        FA = y_all[c]
        WB = py_all[c + 4]
        dot = (FA * WB).sum(axis=0)
        lsf = _log_s(mx_all[c])
        lsb = _log_s(mx_all[c + 4])
        den[64 * c:64 * c + 64] = (np.log(dot) + 2 * TH * C_SHIFT - lsf - lsb)
    return den


def _log_s(mx_flat):
    rows = np.asarray(mx_flat).reshape(-1)[:NRESC * 64].reshape(NRESC, 64)
    s = _pow2_recip(rows)
    return np.log(s.astype(np.float64)).sum(axis=0)


def _loss_np_exact(features, start, end, transitions, confidence, mask, labels):
    """Slow exact fallback (handles arbitrary masks)."""
    f64 = np.float64
    feats = np.swapaxes(features, 0, 1).astype(f64)
    m = np.swapaxes(mask, 0, 1).astype(bool)
    labs = np.swapaxes(np.where(labels == -100, 0, labels), 0, 1).astype(np.int64)
    bs = feats.shape[1]
    bar = np.arange(bs)
    emit = np.take_along_axis(feats, labs[:, :, None], axis=2)[..., 0]
    trs = transitions.astype(f64)[labs[:-1], labs[1:]]
    maskf = m[1:].astype(f64)
    log_num = (start.astype(f64)[labs[0]] + emit[0]
               + ((trs + emit[1:]) * maskf).sum(axis=0))
    seq_lens = m.sum(axis=0) - 1
    log_num = log_num + end.astype(f64)[labs[seq_lens, bar]]
    expT = np.exp(transitions.astype(f64))
    alpha = start.astype(f64)[None, :] + feats[0]
    for t in range(1, feats.shape[0]):
        mm = alpha.max(axis=1, keepdims=True)
        nxt = mm + np.log(np.exp(alpha - mm) @ expT) + feats[t]
        alpha = np.where(m[t][:, None], nxt, alpha)
    ae = alpha + end.astype(f64)[None, :]
    mm = ae.max(axis=1, keepdims=True)
    log_den = mm[:, 0] + np.log(np.exp(ae - mm).sum(axis=1))
    return np.float32(((log_den - log_num) * confidence.astype(f64)).mean())


def kernel(features, start_transitions, end_transitions, transitions,
           confidence, attention_mask, labels):
    features = np.ascontiguousarray(np.asarray(features), dtype=np.float32)
    start = np.asarray(start_transitions, dtype=np.float32)
    end = np.asarray(end_transitions, dtype=np.float32)
    transitions = np.asarray(transitions, dtype=np.float32)
    confidence = np.asarray(confidence, dtype=np.float32)
    mask = np.asarray(attention_mask)
    labels = np.asarray(labels)

    fast_ok = (features.shape == (B, S, L) and bool((mask != 0).all()))
    if fast_ok:
        try:
            res = _run_device(features, start, end, transitions)
            den = _combine(res)
            num = _log_num_host(features, start, end, transitions, labels)
            loss = ((den - num) * confidence.astype(np.float64)).mean()
            return np.float32(loss)
        except Exception:
            import traceback
            traceback.print_exc()
    return _loss_np_exact(features, start, end, transitions, confidence,
                          mask, labels)
